# revision 45
# baseline (speedup 1.0000x reference)
"""Causal multi-head attention (B=4, S=2048, D=1024, H=16) on 8 TRN2 cores.

Sharding: data-parallel over batch (4) x tensor-parallel over head groups (2).
Core c handles batch c//2, heads (c%2)*8 .. (c%2)*8+8.  Each core computes a
partial output y_c = attn_out_c @ Wo[rows of its heads]; the host sums the two
partials per batch and adds the bias.

v4: q/k/v projections run as fp8e4m3 DoubleRow matmuls (0.5 cycles/row,
2 k-tiles per pass) with same-scale hi-lo error compensation: each operand
is split T*s = hi + lo (lo rides in e4m3 subnormals), and one psum group
accumulates hi*hi (4 double-k passes) + hi*lo + lo*hi (8 cross-slot
passes), recovering ~fp16 accuracy at 3/4 of the fp16 PE cost.  Scales
(SQ/SK on q/k, folded into the exp activation scale; SV on v, folded into
the ones-column/denominator) keep every fp8 tensor in the normal range.
Everything downstream of the projections is fp16.  attn@v is transposed:
stationary = exp-scores block [keys, 128 queries], moving = v_aug [keys, 65]
-> out [queries, v|sum] costs 65 PE rows instead of 512 per 128-query tile.
Softmax denominators ride along as the ones-column; normalization is a
reciprocal + one stride-0-broadcast DVE multiply per head.  Normalized heads
assemble in [query, head*64] layout and PE-transpose back to [ad, query] for
the output projection.  Scheduling: head h's scores+exp are emitted before
head h-1's attn@v; the next chunk's projection matmul groups and deferred
output-projection units are interleaved into the head loop as PE filler so
the PE never idles while the ACT engine works through the exp chain.
"""

from collections import deque

import numpy as np

import concourse.bass as bass
import concourse.mybir as mybir
import concourse.tile as tile
from concourse.bacc import Bacc
from concourse.bass import AP
from concourse.bass_utils import run_bass_kernel_spmd

F32 = mybir.dt.float32
F16 = mybir.dt.float16
F8 = mybir.dt.float8e4
DR = mybir.MatmulPerfMode.DoubleRow
EXP = mybir.ActivationFunctionType.Exp

# fp8 projection scales (powers of two; lo term rides the same scale via
# e4m3 subnormals, so hi+lo accumulate in one psum group):
SQ = 512.0           # q = x @ (Wq/sqrt(dh) * SQ)
SK = 64.0            # k = x @ (Wk * SK)
SV = 64.0            # v = x @ (Wv * SV)
EXP_SCALE = 1.0 / (SQ * SK)   # folded into the exp activation
ONES = SV            # ones-column value; denominator absorbs the v scale

B, S, D = 4, 2048, 1024
H, DH = 16, 64
G = 2                # head groups (tensor-parallel factor)
HPC = H // G         # heads per core
AD = HPC * DH        # 512: per-core attention dim
P = 128
NK = D // P          # 8 contraction chunks for the projections
SI = 512             # si (query) chunk width
NCI = S // SI        # 4
VW = DH + 1          # 65: v columns + ones column per head
GB = 2               # kj tiles per exp batch (sc psum = GB banks, x2 bufs)

# scheduling constants (tuned against the TimelineSim cost model):
FILL = [0, 0, 1, 4]     # filler units popped per head in chunk ci
WARM = 6                # p-state warmup matmuls
PREL = [8, 5, 3]        # per-chunk: next-chunk heads whose off-diag scores+exp go early
FLUSH = 1               # filler pops right after each chunk's attn@v flush
KAO = 2                 # attn@v psum double-buffering
KEX = 40                # exp-tile ring depth
KAOQ = 4                # ao_q pool bufs
KY = 6                  # yout pool bufs
EVK = "v"               # k-group eviction engine: v=DVE, p=Pool
EVV = "v"               # v-group eviction engine
EVN = "v"               # ao_q normalization engine
KF3 = "11344556"        # per-head filler pops in the last chunk
KPOPS = "1110"
MASK_ENG = "v"          # mask-mul engine: v=DVE, p=Pool
Q0SPLIT = 3             # chunk-0 q-groups emitted in k-halves (0=off)
KPOS = 0                # where in the head iteration proj pops go (0/1/2)          # projection matmul groups popped per head, per chunk
KPREH = [0, 2, 1, 2]    # pre-computation window start head, per chunk


def _emit(nc, tc, xt, wq, wk, wv, wo, masks, ident, y):
    MENG = nc.gpsimd if MASK_ENG == "p" else nc.vector
    # xt: [2(hi|lo), D, S] fp8; w*: [2(lo|hi), D, AD] fp8
    xt_r = xt.rearrange("two (k p) (n s) -> n p two k s", p=P, s=SI)
    with (
        tc.tile_pool(name="persist", bufs=1) as pp,
        tc.tile_pool(name="qpool", bufs=2) as qpool,
        tc.tile_pool(name="xpool", bufs=2) as xpool,
        tc.tile_pool(name="exp", bufs=KEX) as epool,
        tc.tile_pool(name="aoq", bufs=KAOQ) as aoqp,
        tc.tile_pool(name="small", bufs=4) as spool,
        tc.tile_pool(name="yout", bufs=KY) as yp,
        tc.tile_pool(name="ps_u", bufs=2, space="PSUM") as ps_u,
        tc.tile_pool(name="ps_sc", bufs=2, space="PSUM") as ps_sc,
        tc.tile_pool(name="ps_ao", bufs=KAO, space="PSUM") as ps_ao,
    ):
        wq_sb = pp.tile([P, 2, NK, AD], F8)
        wk_sb = pp.tile([P, 2, NK, AD], F8)
        wv_sb = pp.tile([P, 2, NK, AD], F8)
        wo_sb = pp.tile([P, AD // P, D], F16)
        mask_sb = pp.tile([P, 2, SI], F16)
        id_sb = pp.tile([P, P], F16)
        wq_r = wq.rearrange("two (k p) m -> p two k m", p=P)
        wk_r = wk.rearrange("two (k p) m -> p two k m", p=P)
        wv_r = wv.rearrange("two (k p) m -> p two k m", p=P)

        kts = [pp.tile([P, AD // P, SI], F16, name=f"kt{n}") for n in range(NCI)]
        vts = [pp.tile([P, SI // P, HPC, VW], F16, name=f"vt{n}") for n in range(NCI)]
        aoTs = [pp.tile([P, AD // P, SI], F16, name=f"aoT{n}") for n in range(NCI)]

        def emit_x_load(ci):
            # two 4KB tiles (k-tiles 0-3 / 4-7), each holding hi+lo slots;
            # one DMA per (tile, slot) - dram APs are 3-dim max
            xa = xpool.tile([P, 2, NK // 2, SI], F8, tag="xt", name="xa")
            xb = xpool.tile([P, 2, NK // 2, SI], F8, tag="xt", name="xb")
            for two in range(2):
                nc.sync.dma_start(
                    out=xa[:, two, :, :], in_=xt_r[ci][:, two, 0 : NK // 2, :]
                )
                nc.sync.dma_start(
                    out=xb[:, two, :, :], in_=xt_r[ci][:, two, NK // 2 : NK, :]
                )
            return xa, xb

        def proj_groups(ci, xab):
            # Returns (qt, [closures]) - one closure per PE matmul group so
            # the caller can interleave them with attention work.  Each group
            # is a 3-term fp8 DoubleRow hi-lo accumulation into one psum:
            # 4 double-k hi matmuls + 8 cross (w_lo*x_hi + w_hi*x_lo) matmuls.
            xa, xb = xab
            nk2 = NK // 2

            def xh(j, cols=slice(0, SI)):
                # hi slot, k-pair (2j, 2j+1) -> [P, 2, cols]
                t, o = (xa, 0) if 2 * j + 2 <= nk2 else (xb, nk2)
                return t[:, 0, 2 * j - o : 2 * j + 2 - o, cols]

            def xhl(k, cols=slice(0, SI)):
                # (hi, lo) slots of k-tile k -> [P, 2, cols]
                t = xa if k < nk2 else xb
                return t[:, :, k % nk2, cols]

            qt = qpool.tile([P, AD // P, SI], F16, name="qt")
            groups = []

            def qk_group(dst, w_sb, eng, m, ps=None, k0=0, k1=NK):
                # k-range [k0, k1): start on k0==0, evict+stop on k1==NK, so
                # a group may be emitted in halves (pass the same ps)
                if ps is None:
                    ps = ps_u.tile([P, SI], F32, tag="u", name="psp")
                mc = slice(m * P, (m + 1) * P)
                for j in range(k0 // 2, k1 // 2):
                    nc.tensor.matmul(
                        ps,
                        w_sb[:, 1, 2 * j : 2 * j + 2, mc],
                        xh(j),
                        start=(j == k0 // 2 and k0 == 0),
                        stop=False,
                        perf_mode=DR,
                    )
                for k in range(k0, k1):
                    nc.tensor.matmul(
                        ps,
                        w_sb[:, :, k, mc],
                        xhl(k),
                        start=False,
                        stop=(k == NK - 1),
                        perf_mode=DR,
                    )
                if k1 < NK:
                    return ps
                if eng is nc.scalar:
                    eng.copy(dst[:, m, :], ps)
                else:
                    eng.tensor_copy(dst[:, m, :], ps)
                return None

            def v_group(st):
                ps = ps_u.tile([P, AD], F32, tag="u", name="psv")
                sc = slice(st * P, (st + 1) * P)
                for j in range(NK // 2):
                    nc.tensor.matmul(
                        ps,
                        xh(j, sc),
                        wv_sb[:, 1, 2 * j : 2 * j + 2, :],
                        start=(j == 0),
                        stop=False,
                        perf_mode=DR,
                    )
                for k in range(NK):
                    nc.tensor.matmul(
                        ps,
                        xhl(k, sc),
                        wv_sb[:, :, k, :],
                        start=False,
                        stop=(k == NK - 1),
                        perf_mode=DR,
                    )
                vev = nc.gpsimd if EVV == "p" else nc.vector
                vev.tensor_copy(
                    vts[ci][:, st, :, 0:DH],
                    ps.rearrange("p (h d) -> p h d", d=DH),
                )
                vev.memset(vts[ci][:, st, :, DH : DH + 1], ONES)

            kev = nc.gpsimd if EVK == "p" else nc.vector
            for m in range(AD // P):
                groups.append(lambda m=m: qk_group(qt, wq_sb, nc.vector, m))
            for m in range(AD // P):
                groups.append(lambda m=m: qk_group(kts[ci], wk_sb, kev, m))
            for st in range(SI // P):
                groups.append(lambda st=st: v_group(st))
            return qt, groups, qk_group

        def emit_scores(ci, h, qt, gs, ge):
            # scores^T tiles (kT.T @ qT) + exp + causal masking for head h,
            # for kj-tile groups [gs, ge).  Off-diagonal groups (g < 2*ci)
            # only touch kts of earlier chunks, so they can be emitted one
            # chunk-phase early.
            rb = (h % 2) * 64
            tq = h // 2
            nkj = 4 * ci + 4
            exs = []
            for g0 in range(gs * GB, min(ge * GB, nkj), GB):
                gsz = min(GB, nkj - g0)
                # Each diagonal tile jd writes only its live columns
                # [jd*128, SI) - at fp16 any free size runs at full rate.  The
                # exp below still reads the group-min rectangle; the dead
                # columns hold stale-but-finite psum, are never consumed by
                # attn@v (qtile t only reads columns [t*128,(t+1)*128) with
                # t >= jd), and the mask row 0 shifted by jd*128 is exactly
                # the causal predicate f >= p + jd*128.
                los = []
                for j in range(gsz):
                    jd = g0 + j - 4 * ci
                    los.append((jd * P if jd >= 0 else 0, jd))
                g_lo = min(lo for lo, _ in los)
                scp = ps_sc.tile([P, GB, SI], F32, tag="sc", name="scp")
                for j in range(gsz):
                    kj = g0 + j
                    lo = los[j][0]
                    nc.tensor.matmul(
                        scp[:, j, lo:SI],
                        kts[kj // 4][rb : rb + 64, tq, (kj % 4) * P : (kj % 4 + 1) * P],
                        qt[rb : rb + 64, tq, lo:SI],
                        start=True,
                        stop=True,
                    )
                ex = epool.tile([P, GB, SI], F16, tag="ex", name="ex")
                nc.scalar.activation(
                    ex[:, 0:gsz, g_lo:SI], scp[:, 0:gsz, g_lo:SI], EXP,
                    scale=EXP_SCALE,
                )
                for j in range(gsz):
                    lo, jd = los[j]
                    if jd >= 0:
                        # only the 128-col block at the diagonal (qtile t=jd)
                        # is triangular; blocks t>jd are fully live and never
                        # need masking
                        MENG.tensor_mul(
                            ex[:, j, lo : lo + P],
                            ex[:, j, lo : lo + P],
                            mask_sb[:, 0, 0:P],
                        )
                exs.append(ex)
            return exs

        def emit_attnv_norm(ci, h, exs, ao_q):
            # out[queries, v|sum] accumulation per 128-query tile, then
            # softmax-normalize via reciprocal + stride-0 broadcast multiply.
            ao_ps = ps_ao.tile([P, 4, P], F32, tag="ao", name="ao_ps")
            for t in range(4):
                last = 4 * ci + t
                for kj in range(last + 1):
                    nc.tensor.matmul(
                        ao_ps[:, t, 0:VW],
                        exs[kj // GB][:, kj % GB, t * P : (t + 1) * P],
                        vts[kj // 4][:, kj % 4, h, :],
                        start=(kj == 0),
                        stop=(kj == last),
                    )
            rc = spool.tile([P, 4, 1], F32, tag="rc", name="rc")
            nc.vector.reciprocal(rc, ao_ps[:, 0:4, DH : DH + 1])
            base = rc[:, 0:4, 0:1]
            bc = AP(base.tensor, base.offset, [list(base.ap[0]), list(base.ap[1]), [0, DH]])
            nev = nc.gpsimd if EVN == "p" else nc.vector
            nev.tensor_mul(ao_q[:, 0:4, h, :], ao_ps[:, 0:4, 0:DH], bc)

        def emit_transpose(t, ao_q, aoT, eng=None):
            # [query, ad] -> [ad, query] for one 128-query tile via PE
            # transpose-mode (4 128x128 blocks into one psum bank).
            pst = ps_u.tile([P, 4, 2 * P], F16, tag="u", name="pst")
            for c in range(AD // P):
                nc.tensor.transpose(
                    pst[:, c, 0:P], ao_q[:, t, 2 * c : 2 * c + 2, :], id_sb
                )
            if eng is nc.scalar:
                nc.scalar.copy(aoT[:, 0:4, t * P : (t + 1) * P], pst[:, 0:4, 0:P])
            else:
                nc.vector.tensor_copy(aoT[:, 0:4, t * P : (t + 1) * P], pst[:, 0:4, 0:P])

        def emit_outproj(ci, st, half, split=False, eng=None, pool=None):
            ps3 = (pool or ps_u).tile([P, 512], F32, tag="u" if pool is None else "sc", name="ps3")
            for c in range(AD // P):
                nc.tensor.matmul(
                    ps3,
                    aoTs[ci][:, c, st * P : (st + 1) * P],
                    wo_sb[:, c, half * 512 : (half + 1) * 512],
                    start=(c == 0),
                    stop=(c == AD // P - 1),
                )
            ysb = yp.tile([P, 512], F16, name="ysb")
            rows = y[ci * SI + st * P : ci * SI + (st + 1) * P, :]
            if split:
                # tail latency: copy the two halves on different engines and
                # overlap the two output DMAs on separate queues
                nc.vector.tensor_copy(ysb[:, 0:256], ps3[:, 0:256])
                nc.scalar.copy(ysb[:, 256:512], ps3[:, 256:512])
                for q, qeng in enumerate((nc.sync, nc.sync)):
                    qeng.dma_start(
                        out=rows[:, half * 512 + q * 256 : half * 512 + (q + 1) * 256],
                        in_=ysb[:, q * 256 : (q + 1) * 256],
                    )
            elif eng is nc.scalar:
                nc.scalar.copy(ysb, ps3)
                nc.sync.dma_start(
                    out=rows[:, half * 512 : (half + 1) * 512],
                    in_=ysb,
                )
            else:
                nc.vector.tensor_copy(ysb, ps3)
                nc.sync.dma_start(
                    out=rows[:, half * 512 : (half + 1) * 512],
                    in_=ysb,
                )

        # ---- PE warmup: matmuls on scratch data issued before any DMA so
        # the p-state ramp completes during the startup DMA window --------
        if WARM:
            wsc = pp.tile([P, 256], F16)
            nc.vector.memset(wsc, 0.0)
            wps = ps_u.tile([P, 256], F32, tag="u", name="wps")
            for i in range(WARM):
                nc.tensor.matmul(wps, wsc[:, 0:P], wsc, start=(i == 0), stop=(i == WARM - 1))
        # ---- startup DMAs on four queues (SP/DVE/Pool/ACT are all idle at
        # t=0) so the first projection group's deps land in one DMA round --
        xa0 = xpool.tile([P, 2, NK // 2, SI], F8, tag="xt", name="xa")
        xb0 = xpool.tile([P, 2, NK // 2, SI], F8, tag="xt", name="xb")
        for h2, xt0 in enumerate((xa0, xb0)):
            # k-half-major: all of k 0-3 (hi then lo) lands before k 4-7,
            # matching the Q0SPLIT A-half emission
            nc.sync.dma_start(
                out=xt0[:, 0, :, :],
                in_=xt_r[0][:, 0, 4 * h2 : 4 * h2 + 4, :],
            )
            nc.sync.dma_start(
                out=wq_sb[:, 1, 4 * h2 : 4 * h2 + 4, :],
                in_=wq_r[:, 1, 4 * h2 : 4 * h2 + 4, :],
            )
            nc.sync.dma_start(
                out=xt0[:, 1, :, :],
                in_=xt_r[0][:, 1, 4 * h2 : 4 * h2 + 4, :],
            )
            nc.sync.dma_start(
                out=wq_sb[:, 0, 4 * h2 : 4 * h2 + 4, :],
                in_=wq_r[:, 0, 4 * h2 : 4 * h2 + 4, :],
            )
        for h2 in range(2):
            nc.sync.dma_start(
                out=wk_sb[:, 1, 4 * h2 : 4 * h2 + 4, :],
                in_=wk_r[:, 1, 4 * h2 : 4 * h2 + 4, :],
            )
            nc.sync.dma_start(
                out=wk_sb[:, 0, 4 * h2 : 4 * h2 + 4, :],
                in_=wk_r[:, 0, 4 * h2 : 4 * h2 + 4, :],
            )
        nc.sync.dma_start(out=wv_sb[:, 1, :, :], in_=wv_r[:, 1, :, :])
        nc.sync.dma_start(out=wv_sb[:, 0, :, :], in_=wv_r[:, 0, :, :])
        nc.sync.dma_start(out=mask_sb, in_=masks[:, :, :])
        nc.sync.dma_start(out=id_sb, in_=ident[:, :])
        nc.sync.dma_start(out=wo_sb, in_=wo.rearrange("(t p) m -> p t m", p=P))

        qt0, groups0, qk0 = proj_groups(0, (xa0, xb0))
        if Q0SPLIT:
            # chunk-0 q-groups in k-halves with a psum borrowed from the
            # (still idle) scores pool: PE starts ~3us earlier while the
            # startup DMAs stream in
            nsp = min(Q0SPLIT, 3)
            hps = []
            for m in range(nsp):
                ps = (ps_sc if m == 2 else ps_u).tile(
                    [P, SI], F32, tag=("sc" if m == 2 else "u"), name="psp")
                qk0(qt0, wq_sb, nc.vector, m, ps=ps, k0=0, k1=NK // 2)
                hps.append(ps)
            for m in range(nsp):
                qk0(qt0, wq_sb, nc.vector, m, ps=hps[m], k0=NK // 2, k1=NK)
            for g in groups0[nsp:]:
                g()
        else:
            for g in groups0:
                g()
        qts = {0: qt0}
        ao_qs = {}
        fillers = deque()
        pre_exs = {}
        pend = None  # (ci, h, exs) - carried ACROSS chunk boundaries so the
        # boundary attn@v is emitted after the next chunk's first scores
        for ci in range(NCI):
            ao_qs[ci] = aoqp.tile([P, 4, HPC, DH], F16, tag="aoq", name="ao_q")
            if ci + 1 < NCI:
                qts[ci + 1], pgroups, _ = proj_groups(ci + 1, emit_x_load(ci + 1))
            else:
                pgroups = []
            pgroups = deque(pgroups)
            qt = qts.pop(ci)
            ngrp = 2 * ci + 2
            npopped = 0
            for h in range(HPC):
                if ci >= 1 and h == 1:
                    for t in range(4):
                        fillers.append(
                            lambda t=t, c=ci - 1: emit_transpose(t, ao_qs[c], aoTs[c])
                        )
                    for st in range(4):
                        for half in range(2):
                            fillers.append(
                                lambda st=st, half=half, c=ci - 1: emit_outproj(c, st, half)
                            )
                if KPOS == 0:
                    for _ in range(int(KPOPS[ci])):
                        if pgroups:
                            pgroups.popleft()()
                            npopped += 1
                exs = pre_exs.pop((ci, h), None)
                if exs is None:
                    exs = emit_scores(ci, h, qt, 0, 2 * ci)
                exs = exs + emit_scores(ci, h, qt, 2 * ci, ngrp)
                if KPOS == 1:
                    for _ in range(int(KPOPS[ci])):
                        if pgroups:
                            pgroups.popleft()()
                            npopped += 1
                if pend is not None:
                    emit_attnv_norm(pend[0], pend[1], pend[2], ao_qs[pend[0]])
                pend = (ci, h, exs)
                if KPOS == 2:
                    for _ in range(int(KPOPS[ci])):
                        if pgroups:
                            pgroups.popleft()()
                            npopped += 1
                if ci + 1 < NCI and ci + 1 >= 1 and KPREH[ci] <= h < KPREH[ci] + PREL[ci]:
                    hp = h - KPREH[ci]
                    # the pre-computed scores read qt(ci+1)[:, hp//2, :]; make
                    # sure that projection group has been emitted first
                    while npopped < hp // 2 + 1 and pgroups:
                        pgroups.popleft()()
                        npopped += 1
                    pre_exs[(ci + 1, hp)] = emit_scores(
                        ci + 1, hp, qts[ci + 1], 0, 2 * (ci + 1)
                    )
                npop = FILL[ci]
                if ci == NCI - 1 and len(KF3) == HPC:
                    npop = int(KF3[h])
                for _ in range(npop):
                    if fillers:
                        fillers.popleft()()
            while pgroups:
                pgroups.popleft()()
            for _ in range(FLUSH):
                if fillers:
                    fillers.popleft()()
        emit_attnv_norm(pend[0], pend[1], pend[2], ao_qs[pend[0]])
        for t in range(4):
            emit_transpose(t, ao_qs[NCI - 1], aoTs[NCI - 1],
                           eng=(nc.scalar if t % 2 else nc.vector))
        while fillers:
            fillers.popleft()()
        for st in range(4):
            for half in range(2):
                emit_outproj(NCI - 1, st, half,
                             split=False,
                             eng=(nc.scalar if (2 * st + half) % 2 else nc.vector),
                             pool=(ps_sc if (2 * st + half) % 2 else None))


def build():
    nc = Bacc()
    xt = nc.dram_tensor("xt", [2, D, S], F8, kind="ExternalInput")
    wq = nc.dram_tensor("wq", [2, D, AD], F8, kind="ExternalInput")
    wk = nc.dram_tensor("wk", [2, D, AD], F8, kind="ExternalInput")
    wv = nc.dram_tensor("wv", [2, D, AD], F8, kind="ExternalInput")
    wo = nc.dram_tensor("wo", [AD, D], F16, kind="ExternalInput")
    masks = nc.dram_tensor("masks", [P, 2, SI], F16, kind="ExternalInput")
    ident = nc.dram_tensor("ident", [P, P], F16, kind="ExternalInput")
    y = nc.dram_tensor("y", [S, D], F16, kind="ExternalOutput")
    with tile.TileContext(nc) as tc:
        _emit(nc, tc, xt, wq, wk, wv, wo, masks, ident, y)
    nc.compile()
    return nc


_NC = None


def _causal_masks():
    p = np.arange(P)[:, None]
    f = np.arange(SI)[None, :]
    return np.stack(
        [(f >= p).astype(np.float32), (f >= p + P).astype(np.float32)], axis=1
    )  # [P, 2, SI]


def _split8_w(a, s):
    """[lo, hi] same-scale e4m3 pair of a*s (lo rides in subnormals)."""
    import ml_dtypes

    a32 = np.ascontiguousarray(a, np.float32) * s
    hi = a32.astype(ml_dtypes.float8_e4m3)
    lo = (a32 - hi.astype(np.float32)).astype(ml_dtypes.float8_e4m3)
    return np.stack([lo, hi])


def _split8_x(a):
    """[hi, lo] same-scale e4m3 pair of a."""
    import ml_dtypes

    a32 = np.ascontiguousarray(a, np.float32)
    hi = a32.astype(ml_dtypes.float8_e4m3)
    lo = (a32 - hi.astype(np.float32)).astype(ml_dtypes.float8_e4m3)
    return np.stack([hi, lo])


def run(x, Wq, Wk, Wv, Wo, bo, **run_kwargs):
    global _NC
    x = np.asarray(x, np.float32)
    Wq = np.asarray(Wq, np.float32)
    Wk = np.asarray(Wk, np.float32)
    Wv = np.asarray(Wv, np.float32)
    Wo = np.asarray(Wo, np.float32)
    bo = np.asarray(bo, np.float32)

    if _NC is None:
        _NC = build()

    masks = _causal_masks().astype(np.float16)
    ident = np.eye(P, dtype=np.float16)
    wq_s = Wq * (SQ / np.sqrt(DH))  # fold 1/sqrt(dh) + fp8 scale into q
    in_maps = []
    for c in range(2 * B):
        b, g = divmod(c, G)
        cols = slice(g * AD, (g + 1) * AD)
        in_maps.append(
            {
                "xt": _split8_x(x[b].T),
                "wq": _split8_w(wq_s[:, cols], 1.0),
                "wk": _split8_w(Wk[:, cols], SK),
                "wv": _split8_w(Wv[:, cols], SV),
                "wo": np.ascontiguousarray(Wo[cols, :]).astype(np.float16),
                "masks": masks,
                "ident": ident,
            }
        )

    res = run_bass_kernel_spmd(_NC, in_maps, core_ids=list(range(2 * B)), **run_kwargs)
    ys = [np.asarray(m["y"], np.float32) for m in res.results]
    out = np.stack([ys[G * b] + ys[G * b + 1] for b in range(B)]) + bo
    return out.astype(np.float32), res


def kernel(**inputs):
    out, _ = run(**inputs)
    return out

